# revision 31
# baseline (speedup 1.0000x reference)
"""Cosine-similarity retrieval kernel for Trainium2 (8 NeuronCores, SPMD).

out[q, k] = (z_query[q] . z_support[k]) / (max(||z_query[q]||, eps) * max(||z_support[k]||, eps))

Sharding: z_query split along Q across 8 cores; z_support replicated.
Per core: [1024, 256] x [4096, 256] -> [1024, 4096]  (~21 MB HBM traffic,
memory-bound: roofline ~60 us at ~360 GB/s per-core HBM bandwidth).

Design:
  - fold 1/max(norm, eps) into both operands on-chip, cast to fp16 so the
    PE runs at 1 cycle/row (fp32 would be 4x slower and PE-bound),
  - row norms: one whole-tile Square on ACT + one whole-tile reduce on
    DVE; sqrt on ACT (the Sqrt table set also holds Square/Copy, and a
    dummy sqrt up front makes it the only table load); reciprocal + one
    broadcast multiply (normalize + fp16 cast) on DVE,
  - the z_query path normalizes on ACT (otherwise idle early) in two
    independent half-blocks so the first matmuls only wait on half 0,
  - PE transposes put D on partitions: the nrows transposes of one 128-col
    D-block accumulate in one PSUM bank, then one strided copy scatters
    them into natural column order,
  - fp16 matmuls accumulate D=256 in two 128-chunks into [128, kb] PSUM
    tiles; one PSUM->SBUF copy each, distributed across ACT and DVE,
  - z_support is processed in KSPLIT column blocks so matmul + output DMA
    overlap preprocessing; the first block's chain is the fill-time
    critical path, so its load is issued first.
"""

import sys

for _p in ("/opt/trn_rl_repo", "/opt/pypackages"):
    if _p not in sys.path:
        sys.path.append(_p)

import numpy as np

import concourse.bass as bass
import concourse.bacc as bacc
import concourse.mybir as mybir
import concourse.tile as tile
from concourse.bass_utils import run_bass_kernel_spmd
from concourse.masks import make_identity

Q, D, K = 8192, 256, 4096
NCORES = 8
QL = Q // NCORES  # 1024 query rows per core
P = 128
EPS = 1e-8  # torch F.cosine_similarity default
F32 = mybir.dt.float32

MM_DT = mybir.dt.float16  # matmul operand dtype (1 cycle/row on PE)
# z_support column-block widths: small leading blocks shorten the pipeline
# fill (first output DMA launches after block 0's chain), larger later
# blocks amortize per-instruction overheads.
BLOCKS = (512, 512, 1024, 1024, 1024)
ACT_OF_8 = 6              # of every 8 output copies, this many go to ACT

NQ = QL // P              # 8 query rows per partition


SQUARE_DVE = False


def _bcast(ap, n):
    """Append a step-0 free dim of size n (per-row scalar -> row broadcast)."""
    return bass.AP(tensor=ap.tensor, offset=ap.offset, ap=[*ap.ap, [0, n]])


def _row_normalize(nc, pool, stat, raw, nrows, out_dt, tag, eps2, norm_dve):
    """normed[:, n, :] = raw[:, n, :] / max(||raw[:, n, :]||, EPS), cast to out_dt.

    One whole-tile Square on ACT + one whole-tile reduce on DVE (fewer,
    larger instructions than per-row accumulation; tensor_tensor_reduce
    would do it in one DVE pass but fails in walrus codegen on this
    toolchain).
    """
    sq_full = pool.tile([P, nrows, D], F32, name=f"sqf_{tag}", tag=f"sqf_{tag}")
    if SQUARE_DVE:
        nc.vector.tensor_mul(out=sq_full, in0=raw, in1=raw)
    else:
        nc.scalar.activation(
            out=sq_full, in_=raw, func=mybir.ActivationFunctionType.Square
        )
    sq = stat.tile([P, nrows], F32, name=f"sq_{tag}", tag=f"sq_{tag}")
    nc.vector.reduce_sum(out=sq, in_=sq_full, axis=mybir.AxisListType.X)
    # max(norm, EPS) == sqrt(sumsq + EPS^2) for this data (row norms ~16,
    # EPS=1e-8 never binds) and keeps the divide safe.
    norm = stat.tile([P, nrows], F32, name=f"norm_{tag}", tag=f"norm_{tag}")
    nc.scalar.activation(
        out=norm, in_=sq, func=mybir.ActivationFunctionType.Sqrt, bias=eps2[:, :]
    )
    inv = stat.tile([P, nrows], F32, name=f"inv_{tag}", tag=f"inv_{tag}")
    nc.vector.reciprocal(out=inv, in_=norm)
    normed = pool.tile([P, nrows, D], out_dt, name=f"nrm_{tag}", tag=f"nrm_{tag}")
    if norm_dve:
        nc.vector.tensor_mul(out=normed, in0=raw, in1=_bcast(inv, D))
    else:
        for n in range(nrows):
            nc.scalar.mul(out=normed[:, n, :], in_=raw[:, n, :], mul=inv[:, n : n + 1])
    return normed


def _copy(nc, eng, out, in_):
    if eng is nc.vector:
        nc.vector.tensor_copy(out=out, in_=in_)
    else:
        nc.scalar.copy(out=out, in_=in_)


BATCH_TRANSPOSE = True


def _transpose_blocks(nc, psum_t, ident, src, nrows, dsts, copy_eng):
    """PE-transpose src [P, nrows, D] into dsts[db] [P, nrows*P] (D on partitions).

    Source partition p slot j holds row r = p*nrows + j. For each 128-wide
    D-block db, the nrows transposes accumulate into one PSUM bank
    [P, nrows, P]; one strided copy scatters column p of slot j to dst
    column p*nrows + j (natural row order).
    """
    for db in range(2):
        if BATCH_TRANSPOSE:
            pst = psum_t.tile([P, nrows, P], src.dtype, name="pst", tag="pst")
            for j in range(nrows):
                nc.tensor.transpose(
                    pst[:, j, :], src[:, j, db * P : (db + 1) * P], ident
                )
            dst = dsts[db].rearrange("a (p j) -> a j p", j=nrows)
            _copy(nc, copy_eng, dst, pst)
        else:
            for j in range(nrows):
                pst = psum_t.tile([P, P], src.dtype, name="pst", tag="pst")
                nc.tensor.transpose(pst, src[:, j, db * P : (db + 1) * P], ident)
                dst = dsts[db].rearrange("a (p j) -> a p j", j=nrows)[:, :, j]
                _copy(nc, copy_eng, dst, pst)


def build_nc(mm_dt=MM_DT, blocks=BLOCKS, act_of_8=ACT_OF_8,
             spool_bufs=3, out_bufs=8, f32r=False):
    if f32r:
        mm_dt = F32  # operands stay fp32; matmuls read them as float32r
    assert sum(blocks) == K
    starts = [sum(blocks[:i]) for i in range(len(blocks))]

    nc = bacc.Bacc("TRN2", target_bir_lowering=False, debug=False)
    zq_d = nc.dram_tensor("z_query", [QL, D], F32, kind="ExternalInput").ap()
    zs_d = nc.dram_tensor("z_support", [K, D], F32, kind="ExternalInput").ap()
    out_d = nc.dram_tensor("out", [QL, K], F32, kind="ExternalOutput").ap()

    with tile.TileContext(nc) as tc:
        with (
            tc.tile_pool(name="consts", bufs=1) as consts,
            tc.tile_pool(name="qpool", bufs=1) as qpool,
            tc.tile_pool(name="spool", bufs=spool_bufs) as spool,
            tc.tile_pool(name="tpool", bufs=4) as tpool,
            tc.tile_pool(name="outpool", bufs=out_bufs) as outpool,
            tc.tile_pool(name="stat", bufs=2) as stat,
            tc.tile_pool(name="psum_t", bufs=2, space="PSUM") as psum_t,
            tc.tile_pool(name="psum_mm", bufs=2 if f32r else 3, space="PSUM") as psum_mm,
        ):
            ident = consts.tile([P, P], mm_dt)
            make_identity(nc, ident)
            eps2 = consts.tile([P, 1], F32)
            nc.vector.memset(eps2, EPS * EPS)
            # Dummy sqrt: makes the Sqrt table set (which also contains
            # Square and Copy) the first one loaded, at t~0 under the first
            # input DMA — otherwise the load lands mid-chain before the
            # first real sqrt.
            warm = consts.tile([P, 1], F32)
            nc.scalar.activation(
                out=warm, in_=eps2, func=mybir.ActivationFunctionType.Sqrt
            )

            def prep_zs(i):
                c0, kb = starts[i], blocks[i]
                nsq = kb // P
                zs_raw = spool.tile([P, nsq, D], F32, name="zs_raw", tag="zs_raw")
                nc.sync.dma_start(
                    out=zs_raw,
                    in_=zs_d[c0 : c0 + kb, :].rearrange("(p n) d -> p n d", p=P),
                )
                zs_n = _row_normalize(
                    nc, spool, stat, zs_raw, nsq, mm_dt, "s", eps2, norm_dve=True,
                )
                zsT = [
                    tpool.tile([P, kb], mm_dt, name=f"zsT{db}", tag=f"zsT{db}")
                    for db in range(2)
                ]
                _transpose_blocks(nc, psum_t, ident, zs_n, nsq, zsT, nc.vector)
                return zsT

            # Block 0 feeds the first output DMA: its chain goes first.
            zsT0 = prep_zs(0)

            # z_query path in two independent half-blocks (so the first
            # matmuls only wait on half 0): stats on DVE, the rest on ACT
            # (idle early; keeps the z_support DVE chain unblocked).
            nqh = NQ // 2
            qlh = QL // 2

            def prep_zq(h):
                zq_raw = qpool.tile(
                    [P, nqh, D], F32, name=f"zq_raw{h}", tag=f"zq_raw{h}"
                )
                nc.sync.dma_start(
                    out=zq_raw,
                    in_=zq_d[h * qlh : (h + 1) * qlh, :].rearrange(
                        "(p n) d -> p n d", p=P
                    ),
                )
                zq_n = _row_normalize(
                    nc, qpool, stat, zq_raw, nqh, mm_dt, f"q{h}", eps2, norm_dve=False,
                )
                zqTh = [
                    qpool.tile([P, qlh], mm_dt, name=f"zqT{h}{db}", tag=f"zqT{h}{db}")
                    for db in range(2)
                ]
                _transpose_blocks(nc, psum_t, ident, zq_n, nqh, zqTh, nc.scalar)
                return zqTh

            zqT_half = [prep_zq(0)]

            # ---- matmul + output, interleaved with remaining block preps.
            # The next block's preprocessing is emitted BEFORE this block's
            # matmuls so the Tile scheduler prioritizes it (software
            # pipelining): its chain must complete before this block's
            # output copies drain, or the output-DMA stream starves.
            ncopy = 0
            zsT_next = zsT0
            for i in range(len(blocks)):
                c0, kb = starts[i], blocks[i]
                nb = kb // 512
                zsT = zsT_next
                if i + 1 < len(blocks):
                    zsT_next = prep_zs(i + 1)
                if i == 0:
                    zqT_half.append(prep_zq(1))
                for qb in range(NQ):
                    out_row = outpool.tile([P, kb], F32, name="out_row", tag="out_row")
                    pss = psum_mm.tile([P, kb], F32, name="ps", tag="ps")
                    qh, qs = divmod(qb, nqh)
                    _r = (lambda ap: ap.bitcast(mybir.dt.float32r)) if f32r else (lambda ap: ap)
                    for db in range(2):
                        for b in range(nb):
                            nc.tensor.matmul(
                                pss[:, b * 512 : (b + 1) * 512],
                                lhsT=_r(zqT_half[qh][db][:, qs * P : (qs + 1) * P]),
                                rhs=_r(zsT[db][:, b * 512 : (b + 1) * 512]),
                                start=(db == 0),
                                stop=(db == 1),
                            )
                    eng = nc.scalar if (ncopy % 8) < act_of_8 else nc.vector
                    ncopy += 1
                    _copy(nc, eng, out_row, pss)
                    nc.sync.dma_start(
                        out=out_d[qb * P : (qb + 1) * P, c0 : c0 + kb],
                        in_=out_row,
                    )
    nc.finalize()
    return nc


_NC_CACHE = {}


def _get_nc():
    key = (MM_DT, BLOCKS)
    if key not in _NC_CACHE:
        _NC_CACHE[key] = build_nc()
    return _NC_CACHE[key]


def kernel(z_query: np.ndarray, z_support: np.ndarray) -> np.ndarray:
    z_query = np.ascontiguousarray(np.asarray(z_query, dtype=np.float32))
    z_support = np.ascontiguousarray(np.asarray(z_support, dtype=np.float32))
    assert z_query.shape == (Q, D) and z_support.shape == (K, D)

    nc = _get_nc()
    in_maps = [
        {"z_query": z_query[c * QL : (c + 1) * QL], "z_support": z_support}
        for c in range(NCORES)
    ]
    res = run_bass_kernel_spmd(nc, in_maps, list(range(NCORES)))
    return np.concatenate([res.results[c]["out"] for c in range(NCORES)], axis=0)


if __name__ == "__main__":
    rng = np.random.default_rng(0)
    zq = rng.standard_normal((Q, D), dtype=np.float32)
    zs = rng.standard_normal((K, D), dtype=np.float32)
    out = kernel(zq, zs)
    qn = np.maximum(np.linalg.norm(zq, axis=1), EPS)
    sn = np.maximum(np.linalg.norm(zs, axis=1), EPS)
    ref = (zq @ zs.T) / (qn[:, None] * sn[None, :])
    err = np.linalg.norm(out - ref) / np.linalg.norm(ref)
    print("rel err:", err)
